# revision 3
# baseline (speedup 1.0000x reference)
"""Bidirectional additive (Bahdanau) attention kernel for 8 TRN2 NeuronCores.

Math: score[b,i,j] = sum_a ws[a] * tanh(p1[b,i,a] + p2[b,j,a]) (+ bs, masked),
then softmax over each direction and two weighted sums.

Key trick: tanh(x+y) is approximated by a 12-term Fourier sine series
    tanh(z) ~= sum_r c_r sin(w_r z),   z in [-8, 8]   (max err 1.3e-3)
and sin(w(x+y)) = sin(wx)cos(wy) + cos(wx)sin(wy) is separable, so the whole
[L1, L2, A] tanh grid collapses into one TensorEngine matmul with K = A*2R.
Sin args are range-reduced to [-pi, pi]: the integer quotients
k = round(x*w/2pi) are precomputed on the host (int8 planes, tiny), and the
device does one fused scalar_tensor_tensor pass g = (x*s) - k per feature.
Host/device p-values agree to ~1e-7, so the reduced args stay in-domain.

Sharding: core c = 2*b + h handles batch b and L1-half h (i in [h*256,(h+1)*256)).
The softmax over L2 (w2/o2) is local; the softmax over L1 (w1/o1) needs a
pairwise AllReduce of the per-j exp-sums (2KB) and a pairwise ReduceScatter of
the partial o1 (512KB). No max-subtraction is needed: |score| <= sum|ws| ~ 4.8,
so exp() cannot overflow, and masked entries are -1e30 -> exp gives exactly 0.
"""

import numpy as np

B, L1, L2 = 4, 512, 512
KD, A = 256, 128
VD = 256
LH = L1 // 2          # 256 rows of L1 per core
N_CORES = 8
BIGNEG = -1e30

# Fourier-sine fit of tanh on [-8, 8] (period 2*9.6), max err 1.28e-3.
FREQS = [0.327249235, 0.654498469, 0.981747704, 1.30899694, 1.63624617,
         1.96349541, 2.29074464, 2.61799388, 2.94524311, 3.27249235,
         3.59974158, 3.92699082]
COEFS = [1.23280772, -0.0240609022, 0.321878442, -0.027486926, 0.123802411,
         -0.0170889569, 0.0470001527, -0.00614384173, 0.0152951736,
         -0.000176970962, 0.00365687702, 0.00187698751]
RF = len(FREQS)

_PROGRAM_CACHE = {}


def _build_program():
    import concourse.bass as bass
    import concourse.tile as tile
    import concourse.mybir as mybir
    from concourse import bacc

    AF = mybir.ActivationFunctionType
    ALU = mybir.AluOpType
    F32 = mybir.dt.float32
    I32 = mybir.dt.int32
    TWO_PI = float(2 * np.pi)

    nc = bacc.Bacc("TRN2", debug=False, num_devices=N_CORES)

    # ---- dram parameters (per-core shards; same names on every core) ----
    dp = nc.declare_dram_parameter
    k1T = dp("k1T", [KD, LH], F32, isOutput=False)      # k1[b, ih].T
    k2T = dp("k2T", [KD, L2], F32, isOutput=False)      # k2[b].T
    v1h = dp("v1h", [LH, VD], F32, isOutput=False)      # v1[b, ih]
    v2f = dp("v2f", [L2, VD], F32, isOutput=False)      # v2[b]
    W1d = dp("W1", [KD, A], F32, isOutput=False)
    W2d = dp("W2", [KD, A], F32, isOutput=False)
    b1v = dp("b1v", [A, 1], F32, isOutput=False)
    b2v = dp("b2v", [A, 1], F32, isOutput=False)
    cws = dp("cws", [A, RF], F32, isOutput=False)       # column r = c_r * ws
    maskbs = dp("maskbs", [LH, L2], F32, isOutput=False)  # where(mask,-inf,bs)
    u3d = dp("u3", [3, LH], F32, isOutput=False)        # rank-3 finite mask, i side
    w3d = dp("w3", [3, L2], F32, isOutput=False)        # rank-3 finite mask, j side
    identd = dp("ident", [A, A], F32, isOutput=False)
    I8 = mybir.dt.int8
    k8d = dp("k8", [2 * RF * A, LH + L2], I8, isOutput=False)  # [r,(sin,cos),a, e]

    score_o = dp("score_h", [LH, L2], F32, isOutput=True)
    w2_o = dp("w2_h", [LH, L2], F32, isOutput=True)
    w1_o = dp("w1_h", [L2, LH], F32, isOutput=True)
    o2_o = dp("o2_h", [LH, VD], F32, isOutput=True)
    o1_o = dp("o1_h", [LH, VD], F32, isOutput=True)

    groups = [[2 * b_, 2 * b_ + 1] for b_ in range(B)]

    with tile.TileContext(nc) as tc:
        with (
            tc.tile_pool(name="persist", bufs=1) as P,
            tc.tile_pool(name="feat", bufs=3) as FP,
            tc.tile_pool(name="red", bufs=3) as RP,
            tc.tile_pool(name="stage", bufs=2) as SP,
            tc.tile_pool(name="psA", bufs=1, space="PSUM") as PSA,
            tc.tile_pool(name="psB", bufs=2, space="PSUM") as PSB,
            tc.tile_pool(name="dram", bufs=1, space="DRAM") as DR,
        ):
            # ---------- load inputs ----------
            def load(pool, src, p, f, nm):
                t = pool.tile([p, f], F32, name=nm, tag=nm)
                nc.sync.dma_start(t[:], src[:])
                return t

            def load_tiles(src, n, p, f, nm, rowlen):
                ts = []
                for t_ in range(n):
                    t = P.tile([p, f], F32, name=f"{nm}{t_}", tag=f"{nm}{t_}")
                    nc.sync.dma_start(t[:], src[t_ * p:(t_ + 1) * p, :])
                    ts.append(t)
                return ts

            k1T_t = load_tiles(k1T, 2, A, LH, "k1Tt", LH)
            k2T_t = load_tiles(k2T, 2, A, L2, "k2Tt", L2)
            W1_t = load_tiles(W1d, 2, A, A, "W1t", A)
            W2_t = load_tiles(W2d, 2, A, A, "W2t", A)
            b1_t = load(P, b1v, A, 1, "b1t")
            b2_t = load(P, b2v, A, 1, "b2t")
            cws_t = load(P, cws, A, RF, "cwst")
            ident_t = load(P, identd, A, A, "identt")
            u3_t = load(P, u3d, 3, LH, "u3t")
            w3_t = load(P, w3d, 3, L2, "w3t")
            v1_t = load_tiles(v1h, 2, A, VD, "v1t", VD)
            v2_t = load_tiles(v2f, 4, A, VD, "v2t", VD)
            maskbs_t = load_tiles(maskbs, 2, A, L2, "maskbst", L2)

            # ---------- projections: xa = [p1T + b1 | p2T + b2]  [128, 768] ----------
            xa = P.tile([A, LH + L2], F32, name="xa", tag="xa")
            p1ps = PSA.tile([A, LH], F32, name="p1ps", tag="score0")
            nc.tensor.matmul(p1ps[:], W1_t[0][:], k1T_t[0][:], start=True, stop=False)
            nc.tensor.matmul(p1ps[:], W1_t[1][:], k1T_t[1][:], start=False, stop=True)
            p2ps = PSA.tile([A, L2], F32, name="p2ps", tag="score1")
            nc.tensor.matmul(p2ps[:], W2_t[0][:], k2T_t[0][:], start=True, stop=False)
            nc.tensor.matmul(p2ps[:], W2_t[1][:], k2T_t[1][:], start=False, stop=True)
            nc.vector.tensor_scalar(xa[:, 0:LH], p1ps[:], b1_t[:], None, ALU.add)
            nc.vector.tensor_scalar(xa[:, LH:LH + L2], p2ps[:], b2_t[:], None, ALU.add)

            k8s_t = []
            k8c_t = []
            for r in range(RF):
                ks = P.tile([A, LH + L2], I8, name=f"k8s{r}", tag=f"k8s{r}")
                nc.sync.dma_start(ks[:], k8d[(2 * r) * A:(2 * r + 1) * A, :])
                kc = P.tile([A, LH + L2], I8, name=f"k8c{r}", tag=f"k8c{r}")
                nc.sync.dma_start(kc[:], k8d[(2 * r + 1) * A:(2 * r + 2) * A, :])
                k8s_t.append(ks)
                k8c_t.append(kc)
            halfpi = P.tile([A, 1], F32, name="halfpi", tag="halfpi")
            nc.vector.memset(halfpi[:], float(np.pi / 2))

            # ---------- score psum tiles, accumulated over 2*RF+1 matmuls ----------
            score_ps = [PSA.tile([A, L2], F32, name=f"score_ps{i_}", tag=f"score{i_}")
                        for i_ in range(2)]

            for r in range(RF):
                s = float(FREQS[r] / TWO_PI)
                # g = (x*s) - k  in [-0.5, 0.5); one fused DVE pass per trig kind
                g_r = RP.tile([A, LH + L2], F32, tag="gred")
                nc.vector.scalar_tensor_tensor(g_r[:], xa[:], s, k8s_t[r][:],
                                               ALU.mult, ALU.subtract)
                gc_r = RP.tile([A, LH + L2], F32, tag="gcred")
                nc.vector.scalar_tensor_tensor(gc_r[:], xa[:], s, k8c_t[r][:],
                                               ALU.mult, ALU.subtract)
                sin_r = FP.tile([A, LH + L2], F32, tag="sin")
                nc.scalar.activation(sin_r[:], g_r[:], AF.Sin, scale=TWO_PI)
                # cos(wx) = sin(2pi*(t+0.25-kc)) = sin(2pi*gc + pi/2)
                cos_r = FP.tile([A, LH + L2], F32, tag="cos")
                nc.scalar.activation(cos_r[:], gc_r[:], AF.Sin, bias=halfpi[:], scale=TWO_PI)
                # scale x-side features by c_r * ws (per-partition vector)
                fs = FP.tile([A, LH], F32, tag="fs")
                nc.vector.tensor_scalar(fs[:], sin_r[:, 0:LH], cws_t[:, r:r + 1], None, ALU.mult)
                fc = FP.tile([A, LH], F32, tag="fc")
                nc.vector.tensor_scalar(fc[:], cos_r[:, 0:LH], cws_t[:, r:r + 1], None, ALU.mult)
                for it in range(2):
                    st = (r == 0)
                    nc.tensor.matmul(score_ps[it][:], fs[:, it * A:(it + 1) * A],
                                     cos_r[:, LH:LH + L2], start=st, stop=False)
                    nc.tensor.matmul(score_ps[it][:], fc[:, it * A:(it + 1) * A],
                                     sin_r[:, LH:LH + L2], start=False, stop=False)
            # finite additive mask (rank 3): -C*u_i - C*w_j + 2C*u_i*w_j
            for it in range(2):
                nc.tensor.matmul(score_ps[it][:], u3_t[:, it * A:(it + 1) * A],
                                 w3_t[:], start=False, stop=True)

            # ---------- exp + row stats (no max subtraction needed) ----------
            E2 = [P.tile([A, L2], F32, name=f"E2_{i_}", tag=f"E2_{i_}")
                  for i_ in range(2)]                               # [i, j] layout
            S2 = P.tile([A, 2], F32, name="S2", tag="S2")
            for it in range(2):
                nc.scalar.activation(E2[it][:], score_ps[it][:], AF.Exp,
                                     accum_out=S2[:, it:it + 1])
            # score output = masked psum + (bs or -inf)
            for it in range(2):
                so = SP.tile([A, L2], F32, tag="so")
                nc.vector.tensor_tensor(so[:], score_ps[it][:], maskbs_t[it][:], ALU.add)
                nc.sync.dma_start(score_o[it * A:(it + 1) * A, :], so[:])

            r2 = P.tile([A, 2], F32, name="r2", tag="r2")
            nc.vector.reciprocal(r2[:], S2[:])
            # w2 = E2 / S2
            for it in range(2):
                w2s = SP.tile([A, L2], F32, tag="w2s")
                nc.vector.tensor_scalar(w2s[:], E2[it][:], r2[:, it:it + 1], None, ALU.mult)
                nc.scalar.dma_start(w2_o[it * A:(it + 1) * A, :], w2s[:])

            # ---------- transpose E2 -> E1T [j, i] ----------
            E1T = [P.tile([A, LH], F32, name=f"E1T_{j_}", tag=f"E1T_{j_}")
                   for j_ in range(4)]
            for jt in range(4):
                for it in range(2):
                    tp = PSB.tile([A, A], F32, tag="tp")
                    nc.tensor.transpose(tp[:], E2[it][:, jt * A:(jt + 1) * A], ident_t[:])
                    eng = nc.vector if (jt + it) % 2 == 0 else nc.scalar
                    if eng is nc.vector:
                        nc.vector.tensor_copy(E1T[jt][:, it * A:(it + 1) * A], tp[:])
                    else:
                        nc.scalar.copy(E1T[jt][:, it * A:(it + 1) * A], tp[:])

            # S1 partial = row sums of E1T; AllReduce with pair core
            S1loc = P.tile([A, 4], F32, name="S1loc", tag="S1loc")
            for jt in range(4):
                nc.vector.tensor_reduce(S1loc[:, jt:jt + 1], E1T[jt][:],
                                        mybir.AxisListType.X, ALU.add)
            s1in = DR.tile([A, 4], F32, name="s1in", tag="s1in")
            s1out = DR.tile([A, 4], F32, name="s1out", tag="s1out")
            nc.gpsimd.dma_start(s1in[:], S1loc[:])
            nc.gpsimd.collective_compute(
                "AllReduce", ALU.add, replica_groups=groups,
                ins=[s1in.opt()], outs=[s1out.opt()])
            S1 = P.tile([A, 4], F32, name="S1", tag="S1")
            nc.gpsimd.dma_start(S1[:], s1out[:])
            r1 = P.tile([A, 4], F32, name="r1", tag="r1")
            nc.vector.reciprocal(r1[:], S1[:])

            # w1 = E1T / S1  (full S1 after allreduce)
            for jt in range(4):
                w1s = SP.tile([A, LH], F32, tag="w1s")
                nc.vector.tensor_scalar(w1s[:], E1T[jt][:], r1[:, jt:jt + 1], None, ALU.mult)
                nc.scalar.dma_start(w1_o[jt * A:(jt + 1) * A, :], w1s[:])

            # ---------- o2 = (E1T^T as lhsT) @ v2 / S2 ----------
            for it in range(2):
                o2ps = PSB.tile([A, VD], F32, name="o2ps", tag="o2ps", bufs=1)
                for jt in range(4):
                    nc.tensor.matmul(o2ps[:], E1T[jt][:, it * A:(it + 1) * A],
                                     v2_t[jt][:], start=(jt == 0), stop=(jt == 3))
                o2s = SP.tile([A, VD], F32, tag="o2s")
                nc.vector.tensor_scalar(o2s[:], o2ps[:], r2[:, it:it + 1], None, ALU.mult)
                nc.sync.dma_start(o2_o[it * A:(it + 1) * A, :], o2s[:])

            # ---------- o1 partial = E2 as lhsT @ v1, scaled by 1/S1, ReduceScatter ----------
            ob1 = DR.tile([L2, VD], F32, name="ob1", tag="ob1")
            for jt in range(4):
                o1ps = PSB.tile([A, VD], F32, name="o1ps", tag="o1ps", bufs=1)
                for it in range(2):
                    nc.tensor.matmul(o1ps[:], E2[it][:, jt * A:(jt + 1) * A],
                                     v1_t[it][:], start=(it == 0), stop=(it == 1))
                o1s = SP.tile([A, VD], F32, tag="o1s")
                nc.vector.tensor_scalar(o1s[:], o1ps[:], r1[:, jt:jt + 1], None, ALU.mult)
                nc.gpsimd.dma_start(ob1[jt * A:(jt + 1) * A, :], o1s[:])
            or1 = DR.tile([LH, VD], F32, name="or1", tag="or1")
            nc.gpsimd.collective_compute(
                "ReduceScatter", ALU.add, replica_groups=groups,
                ins=[ob1.opt()], outs=[or1.opt()])
            nc.gpsimd.dma_start(o1_o[:], or1[:])

    nc.compile()
    return nc


def _get_program():
    if "nc" not in _PROGRAM_CACHE:
        _PROGRAM_CACHE["nc"] = _build_program()
    return _PROGRAM_CACHE["nc"]


def _prep_inputs(k1, k2, v1, v2, W1, b1, W2, b2, ws, bs, k1_lengths, k2_lengths):
    f32 = np.float32
    cws_np = (np.asarray(ws, f32)[:, None] * np.asarray(COEFS, f32)[None, :]).astype(f32)
    # host copies of the projections (match device fp32 to ~1e-7; only the
    # integer quotients k are derived from these, and an off-by-eps there
    # shifts the Sin arg by ~2pi*eps which is harmless)
    p1h = [(np.asarray(k1[b_], f32) @ np.asarray(W1, f32) + np.asarray(b1, f32)).astype(f32)
           for b_ in range(B)]
    p2h = [(np.asarray(k2[b_], f32) @ np.asarray(W2, f32) + np.asarray(b2, f32)).astype(f32)
           for b_ in range(B)]
    scales = (np.asarray(FREQS, f32) / f32(2 * np.pi)).astype(f32)
    in_maps = []
    for c in range(N_CORES):
        b_, h_ = c // 2, c % 2
        sl = slice(h_ * LH, (h_ + 1) * LH)
        # xa layout on device: [A, LH + L2] = [p1T half | p2T]
        xa_h = np.concatenate([p1h[b_][sl].T, p2h[b_].T], axis=1).astype(f32)  # [A, LH+L2]
        k8 = np.empty((2 * RF * A, LH + L2), np.int8)
        for r in range(RF):
            t = (xa_h * scales[r]).astype(f32)
            k8[(2 * r) * A:(2 * r + 1) * A] = np.rint(t).astype(np.int8)
            k8[(2 * r + 1) * A:(2 * r + 2) * A] = np.rint(t + f32(0.25)).astype(np.int8)
        u = (np.arange(L1)[sl] >= int(k1_lengths[b_])).astype(f32)   # [LH]
        w = (np.arange(L2) >= int(k2_lengths[b_])).astype(f32)       # [L2]
        C = f32(-BIGNEG)  # 1e30
        u3 = np.stack([-C * u, -C * np.ones_like(u), 2 * C * u]).astype(f32)
        w3 = np.stack([np.ones_like(w), w, w]).astype(f32)
        mask = (u[:, None] + w[None, :]) == 1.0
        maskbs_np = np.where(mask, f32(-np.inf), f32(bs)).astype(f32)
        in_maps.append({
            "k1T": np.ascontiguousarray(np.asarray(k1[b_, sl], f32).T),
            "k2T": np.ascontiguousarray(np.asarray(k2[b_], f32).T),
            "v1h": np.ascontiguousarray(np.asarray(v1[b_, sl], f32)),
            "v2f": np.ascontiguousarray(np.asarray(v2[b_], f32)),
            "W1": np.ascontiguousarray(np.asarray(W1, f32)),
            "W2": np.ascontiguousarray(np.asarray(W2, f32)),
            "b1v": np.ascontiguousarray(np.asarray(b1, f32)[:, None]),
            "b2v": np.ascontiguousarray(np.asarray(b2, f32)[:, None]),
            "cws": cws_np,
            "maskbs": maskbs_np,
            "u3": np.ascontiguousarray(u3),
            "w3": np.ascontiguousarray(w3),
            "ident": np.eye(A, dtype=f32),
            "k8": k8,
        })
    return in_maps


def _execute(inputs, trace=False):
    from concourse.bass_utils import run_bass_kernel_spmd
    nc = _get_program()
    in_maps = _prep_inputs(**inputs)
    res = run_bass_kernel_spmd(nc, in_maps, list(range(N_CORES)), trace=trace)
    f32 = np.float32
    o1 = np.empty((B, L2, VD), f32)
    o2 = np.empty((B, L1, VD), f32)
    w1 = np.empty((B, L2, L1), f32)
    w2 = np.empty((B, L1, L2), f32)
    score = np.empty((B, L1, L2), f32)
    for c in range(N_CORES):
        b_, h_ = c // 2, c % 2
        sl = slice(h_ * LH, (h_ + 1) * LH)
        r = res.results[c]
        score[b_, sl] = r["score_h"]
        w2[b_, sl] = r["w2_h"]
        o2[b_, sl] = r["o2_h"]
        w1[b_, :, sl] = r["w1_h"]
        o1[b_, sl.start:sl.stop] = r["o1_h"]  # RS shard h_ covers j-half h_
    return (o1, o2, w1, w2, score), res


def kernel(**inputs):
    outs, _ = _execute(inputs, trace=False)
    return outs
